# revision 33
# baseline (speedup 1.0000x reference)
"""Trainium2 Bass kernel for causal self-attention (nn_Casualselfatt).

Reference computes (B=2, S=2048, E=1024, H=16, D=64, fp32):
    qkv = x @ W_qkv + b_qkv ; q,k,v = split(qkv)
    q = q.reshape(B, H, S, D)   # NOTE: raw reshape, no transpose.
    ...causal softmax attention per (b,h)...
    out = res @ W_proj + b_proj

The raw reshape means head h of batch b attends over the [S, D] reshape of
rows [128h, 128h+128) of q/k/v[b].  Sharding: 32 (b,h) pairs -> 4 heads of
one batch per core (core c: b=c//4, heads 4*(c%4)..+4).  Each core computes
a partial projection output; the host sums 4 partials per batch.

On-chip: scores are built transposed ([k-part, q-free]) so the softmax
denominator rides an appended ones-column through the AV matmul.  Q/K/V
sequence positions are sigma-permuted within each 128-block (slot =
64*par + 8*mm + rho for t = 16*rho + 2*mm + par) which makes the psum->SBUF
distribution copies contiguous-run; the host un-permutes output rows.

v3+ (engine-overlap restructure, from the v2 trace; ~220us -> ~179us):
  * K/Q weight passes are split by head-pair (N=256) so head-pair 0's
    score/exp stream starts after 16 half-chunks instead of 16 full ones.
  * One long interleaved emission keeps the PE dense (less HAM
    re-throttle): hp0 scores ride with hp1 K/Q halves, V chunks and V
    transposes; hp0 AVs ride with hp1 scores; proj rides with hp1 AVs.
  * V is stored as vt2[128(two heads x 64d), pair, S] so one 128x128 PE
    transpose + ONE [128,(2,64)] copy produce V-natural for TWO heads
    (32 transposes / 32 copies, not 64 / 64).
  * Softmax denominators: reciprocal_approx_fast (input staged to
    partition base 0 - the custom DVE op mis-reads base-64 APs) replaces
    the 3.3us lane-serial DVE reciprocal; broadcast on GpSimd.  GpSimd
    gets NOTHING else: gp tensor ops in the exp->AV or norm path convoy
    all four engine FIFOs (measured +60 to +160us).
  * exp tile ring 48 deep: hp0's ~34 live exp tiles otherwise exhaust
    the ring and stall the ACT exp stream on AV progress.
  * Input DMAs: x/triu first, wqkv streamed, ident/Wproj late.
"""

import os
from collections import deque

import numpy as np
import ml_dtypes

import concourse.bass as bass
import concourse.tile as tile
from concourse import bacc, mybir
import concourse.bass_utils as bass_utils

DEBUG_TAPS = os.environ.get("KERNEL_DEBUG_TAPS") == "1"

F32 = mybir.dt.float32
BF16 = mybir.dt.bfloat16

B, S, E = 2, 2048, 1024
H, D = 16, 64
N_CORES = 8
HEADS_PER_CORE = 4
ROWS = 128 * HEADS_PER_CORE  # x rows per core
NM = 24                      # qkv column chunks of 128 (q:0-7, k:8-15, v:16-23)
KT = 8                       # contraction tiles over E
NG = 4                       # q groups of 512
NB = S // 128                # 16 blocks of 128 along s'


def slot_perm():
    """perm[slot] = t: original within-block position stored at `slot`."""
    p = np.zeros(128, dtype=np.int64)
    for slot in range(128):
        par, rem = divmod(slot, 64)
        mm, rho = divmod(rem, 8)
        p[slot] = 16 * rho + 2 * mm + par
    return p


def build_program(with_qkv_bias: bool):
    nc = bacc.Bacc("TRN2", target_bir_lowering=False, debug=False,
                   num_devices=N_CORES)

    xt_in = nc.dram_tensor("xt", [128, KT, ROWS], BF16, kind="ExternalInput")
    wqkv = nc.dram_tensor("wqkv", [NM, 128, KT, 128], BF16, kind="ExternalInput")
    wproj = nc.dram_tensor("wproj", [2, 128, E], BF16, kind="ExternalInput")
    ident_in = nc.dram_tensor("ident", [128, 128], BF16, kind="ExternalInput")
    triu_in = nc.dram_tensor("triu", [128, 128], BF16, kind="ExternalInput")
    if with_qkv_bias:
        bqkv = nc.dram_tensor("bqkv", [128, NM], F32, kind="ExternalInput")
    out = nc.dram_tensor("out", [S, E], BF16, kind="ExternalOutput")
    if DEBUG_TAPS:
        dbg_qt = nc.dram_tensor("dbg_qt", [128, 2, S], BF16,
                                kind="ExternalOutput")
        dbg_kt = nc.dram_tensor("dbg_kt", [128, 2, S], BF16,
                                kind="ExternalOutput")
        dbg_vt = nc.dram_tensor("dbg_vt", [128, 2, S], BF16,
                                kind="ExternalOutput")
        dbg_res = nc.dram_tensor("dbg_res", [2, 128, S], BF16,
                                 kind="ExternalOutput")

    with tile.TileContext(nc) as tc:
        with (
            tc.tile_pool(name="const", bufs=1) as constp,
            tc.tile_pool(name="persist", bufs=1) as persist,
        ):
            # xT[p, kt, r]: x rows (4 heads * 128) transposed, bf16 (host).
            # Two slabs so the first QKV matmuls can start ~2us earlier.
            xT = persist.tile([128, KT, ROWS], BF16, tag="xT")
            nc.sync.dma_start(xT[:, 0:2, :], xt_in.ap()[:, 0:2, :])

            ident = constp.tile([128, 128], BF16)
            triu = constp.tile([128, 128], BF16)
            wp_sb = [persist.tile([128, E], BF16, tag=f"wp{i}", name=f"wp{i}")
                     for i in range(2)]
            if with_qkv_bias:
                bias_sb = constp.tile([128, NM], F32)
                nc.sync.dma_start(bias_sb[:], bqkv.ap())

            # Q/K transposed: [128 (2 pos x 64 d), hp, 2048 (sigma-slot)]
            qt = persist.tile([128, 2, S], BF16, tag="qt")
            kt_ = persist.tile([128, 2, S], BF16, tag="kt")
            # V transposed: [128 (2 heads x 64 d), pair, 2048 (sigma-slot)]
            vt2 = persist.tile([128, 2, S], BF16, tag="vt2")
            # V natural per head PAIR: 16 blocks of [128, 130] = two heads'
            # [65] slabs (col 64 of each = ones), k rows sigma-permuted
            # within each block.  One [128,128] copy fills both heads.
            vnatp = [persist.tile([128, NB * 130], BF16, tag=f"vn{i}",
                                  name=f"vn{i}") for i in range(2)]
            # res^T per head-pair (normalized), bf16
            res = [persist.tile([128, S], BF16, tag=f"res{i}", name=f"res{i}")
                   for i in range(2)]

            with (
                tc.tile_pool(name="wch", bufs=6) as wch,
                tc.tile_pool(name="qkvps", bufs=2, space="PSUM") as qkvps,
                tc.tile_pool(name="scps", bufs=2, space="PSUM") as scps,
                tc.tile_pool(name="avps", bufs=2, space="PSUM") as avps,
                tc.tile_pool(name="expp", bufs=48) as expp,
                tc.tile_pool(name="nrm", bufs=2) as nrm,
                tc.tile_pool(name="osb", bufs=3) as osb,
            ):
                def kq_half(m, hp, w=None):
                    """K or Q chunk m, head-pair hp only (256 x rows).

                    hp1 re-DMAs the chunk (the wch ring is only 6 deep and
                    hp1 runs ~30us after hp0; +4MB HBM is free next to the
                    engine budgets)."""
                    if w is None:
                        w = wch.tile([128, KT, 128], BF16, tag="w",
                                     name=f"w{m}_{hp}")
                        nc.sync.dma_start(w[:], wqkv.ap()[m])
                    r0 = 256 * hp
                    ps = qkvps.tile([128, 256], F32, tag="ps",
                                    name=f"kq{m}_{hp}")
                    for j in range(KT):
                        nc.tensor.matmul(ps[:], w[:, j, :],
                                         xT[:, j, r0:r0 + 256],
                                         start=(j == 0), stop=(j == KT - 1))
                    dest, mm = (qt, m) if m < 8 else (kt_, m - 8)
                    off = 8 * mm
                    src4 = ps[:].rearrange("p (pos blk rho) -> p pos blk rho",
                                           pos=2, rho=8)
                    dv = dest[:].rearrange("p hp (blk s) -> p hp blk s", s=128)
                    for par in range(2):
                        for pos in range(2):
                            src = src4[64 * par:64 * par + 64, pos]
                            dst = dv[64 * pos:64 * pos + 64, hp, :,
                                     64 * par + off:64 * par + off + 8]
                            if with_qkv_bias:
                                nc.vector.tensor_scalar_add(
                                    dst, src,
                                    bias_sb[64 * par:64 * par + 64, m:m + 1])
                            elif par == 0 or hp == 1:
                                nc.vector.tensor_copy(dst, src)
                            else:
                                nc.scalar.copy(dst, src)

                def v_chunk(m):
                    """V chunk m (full width), into vt2 layout."""
                    w = wch.tile([128, KT, 128], BF16, tag="w", name=f"w{m}")
                    nc.sync.dma_start(w[:], wqkv.ap()[m])
                    ps = qkvps.tile([128, ROWS], F32, tag="ps", name=f"v{m}")
                    for j in range(KT):
                        nc.tensor.matmul(ps[:], w[:, j, :], xT[:, j, :],
                                         start=(j == 0), stop=(j == KT - 1))
                    off = 8 * (m - 16)
                    # r = 256*kap + 128*alpha + 8*blk + rho  (head = 2kap+alpha)
                    src5 = ps[:].rearrange(
                        "p (kap alpha blk rho) -> p kap alpha blk rho",
                        kap=2, alpha=2, rho=8)
                    dv = vt2[:].rearrange("p kap (blk s) -> p kap blk s", s=128)
                    for par in range(2):
                        for alpha in range(2):
                            src = src5[64 * par:64 * par + 64, :, alpha]
                            dst = dv[64 * alpha:64 * alpha + 64, :, :,
                                     64 * par + off:64 * par + off + 8]
                            if with_qkv_bias:
                                nc.vector.tensor_scalar_add(
                                    dst, src,
                                    bias_sb[64 * par:64 * par + 64, m:m + 1])
                            else:
                                nc.vector.tensor_copy(dst, src)

                def vtrans2(kap, jj):
                    """One 128x128 PE transpose -> V-natural for heads
                    2kap and 2kap+1, block jj (single copy fills both)."""
                    vp = qkvps.tile([128, 128], BF16, tag="ps",
                                    name=f"vp{kap}_{jj}")
                    nc.tensor.transpose(
                        vp[:], vt2[:, kap, 128 * jj:128 * jj + 128], ident[:])
                    dst = vnatp[kap][:].rearrange(
                        "p (jj a c) -> p jj a c", a=2, c=65)
                    nc.vector.tensor_copy(
                        dst[:, jj, :, 0:64],
                        vp[:].rearrange("p (a c) -> p a c", a=2))

                def vnat_ones(kap):
                    nc.vector.memset(
                        vnatp[kap][:].rearrange(
                            "p (jj a c) -> p jj a c", a=2, c=65)[:, :, :, 64],
                        1.0)

                def score_block(hp, g, kb):
                    """scores + exp (+ diag mask) for one 128-k block of a
                    512-q group; returns the bf16 exp tile."""
                    q0 = 512 * g
                    ingroup = kb >= 4 * g
                    coff = 128 * (kb - 4 * g) if ingroup else 0
                    sc = scps.tile([128, 1024], F32, tag="sc")
                    ex = expp.tile([128, 1024], BF16, tag="ex")
                    for pos in range(2):
                        so = 512 * pos
                        nc.tensor.matmul(
                            sc[:, so + coff:so + 512],
                            kt_[64 * pos:64 * pos + 64, hp,
                                128 * kb:128 * kb + 128],
                            qt[64 * pos:64 * pos + 64, hp,
                               q0 + coff:q0 + 512],
                            start=True, stop=True,
                            tile_position=(64 * pos, 0))
                    if not ingroup:
                        nc.scalar.activation(
                            ex[:], sc[:],
                            mybir.ActivationFunctionType.Exp,
                            scale=0.125)
                    else:
                        sc3 = sc[:].rearrange("p (s q) -> p s q", s=2)
                        ex3 = ex[:].rearrange("p (s q) -> p s q", s=2)
                        nc.scalar.activation(
                            ex3[:, :, coff:512],
                            sc3[:, :, coff:512],
                            mybir.ActivationFunctionType.Exp,
                            scale=0.125)
                        for pos in range(2):
                            so = 512 * pos
                            nc.vector.tensor_mul(
                                ex[:, so + coff:so + coff + 128],
                                ex[:, so + coff:so + coff + 128],
                                triu[:])
                    return ex

                def av_block(hp, g, kb, ex, av, nkb):
                    ingroup = kb >= 4 * g
                    coff = 128 * (kb - 4 * g) if ingroup else 0
                    for pos in range(2):
                        so = 512 * pos
                        nc.tensor.matmul(
                            av[pos][:, coff:512],
                            vnatp[hp][:, 130 * kb + 65 * pos:
                                       130 * kb + 65 * pos + 65],
                            ex[:, so + coff:so + 512],
                            start=(kb == 0), stop=(kb == nkb - 1))

                def norm_g(hp, g, av, tail=False):
                    """Normalize group g of head-pair hp into res[hp],
                    staging the AV accumulators to SBUF so the psum slots
                    free immediately (avps ring is only 2 deep).

                    dens staged to partition base 0: reciprocal_approx_fast
                    mis-reads inputs whose AP starts at partition 64."""
                    q0 = 512 * g
                    avs = [nrm.tile([64, 512], F32, tag=f"avs{i}",
                                    name=f"avs{hp}_{g}_{i}") for i in range(2)]
                    den = [nrm.tile([1, 512], F32, tag=f"den{i}",
                                    name=f"den{hp}_{g}_{i}") for i in range(2)]
                    nc.vector.tensor_copy(avs[0][:], av[0][0:64, :])
                    nc.vector.tensor_copy(den[0][:], av[0][64:65, :])
                    nc.scalar.copy(avs[1][:], av[1][0:64, :])
                    nc.scalar.copy(den[1][:], av[1][64:65, :])
                    for pos in range(2):
                        rec = nrm.tile([1, 512], F32, tag=f"rec{pos}",
                                       name=f"rec{hp}_{g}_{pos}")
                        nc.vector.reciprocal_approx_fast(rec[:], den[pos][:])
                        bc = nrm.tile([64, 512], F32, tag=f"bc{pos}",
                                      name=f"bc{hp}_{g}_{pos}")
                        nc.gpsimd.partition_broadcast(bc[:], rec[:])
                        nc.vector.tensor_mul(
                            res[hp][64 * pos:64 * pos + 64, q0:q0 + 512],
                            avs[pos][:], bc[:])

                def norm_tail_chain(hp, g, av):
                    """Copies + reciprocal + broadcast for the final group
                    (muls deferred so proj(g-1) can fill the PE meanwhile)."""
                    avs = [nrm.tile([64, 512], F32, tag=f"avs{i}",
                                    name=f"avsT{hp}_{g}_{i}") for i in range(2)]
                    den = [nrm.tile([1, 512], F32, tag=f"den{i}",
                                    name=f"denT{hp}_{g}_{i}") for i in range(2)]
                    nc.vector.tensor_copy(avs[0][:], av[0][0:64, :])
                    nc.vector.tensor_copy(den[0][:], av[0][64:65, :])
                    nc.scalar.copy(avs[1][:], av[1][0:64, :])
                    nc.scalar.copy(den[1][:], av[1][64:65, :])
                    bcs = []
                    for pos in range(2):
                        rec = nrm.tile([1, 512], F32, tag=f"rec{pos}",
                                       name=f"recT{hp}_{g}_{pos}")
                        nc.vector.reciprocal_approx_fast(rec[:], den[pos][:])
                        bc = nrm.tile([64, 512], F32, tag=f"bc{pos}",
                                      name=f"bcT{hp}_{g}_{pos}")
                        nc.gpsimd.partition_broadcast(bc[:], rec[:])
                        bcs.append(bc)
                    return avs, bcs

                def norm_tail_muls(hp, g, avs, bcs):
                    """Per-128-column muls, each releasing its proj block
                    immediately (shortens the endgame critical chain)."""
                    q0 = 512 * g
                    for i, blk in enumerate(range(4 * g, 4 * g + 4)):
                        co = 128 * i
                        for pos in range(2):
                            nc.vector.tensor_mul(
                                res[hp][64 * pos:64 * pos + 64,
                                        q0 + co:q0 + co + 128],
                                avs[pos][:, co:co + 128],
                                bcs[pos][:, co:co + 128])
                        proj_blk(blk, "split")

                def proj_blk(blk, o_eng):
                        o = osb.tile([128, E], BF16, name=f"o{blk}", tag="o")
                        for f in range(2):
                            pp = qkvps.tile([128, 512], F32, tag="ps",
                                            name=f"pp{blk}_{f}")
                            for hp in range(2):
                                nc.tensor.matmul(
                                    pp[:], res[hp][:, 128 * blk:128 * blk + 128],
                                    wp_sb[hp][:, 512 * f:512 * f + 512],
                                    start=(hp == 0), stop=(hp == 1))
                            dst = o[:, 512 * f:512 * f + 512]
                            if o_eng == "act":
                                nc.scalar.copy(dst, pp[:])
                            elif o_eng == "split":
                                (nc.scalar.copy if f else
                                 nc.vector.tensor_copy)(dst, pp[:])
                            else:
                                nc.vector.tensor_copy(dst, pp[:])
                        nc.sync.dma_start(
                            out.ap()[128 * blk:128 * blk + 128, :], o[:])

                # ---- emission ----------------------------------------
                blocks = [(g, kb) for g in range(NG) for kb in range(4 * g + 4)]
                ex0, ex1 = {}, {}

                # A: K then Q for head-pair 0 (scores for hp0 unlocked).
                # DMA order: x slab 0, w8 (the first matmuls' needs), then
                # the rest of x, triu (first exp's mask), remaining chunks.
                w8 = wch.tile([128, KT, 128], BF16, tag="w", name="w8_0")
                nc.sync.dma_start(w8[:], wqkv.ap()[8])
                nc.sync.dma_start(xT[:, 2:KT, :], xt_in.ap()[:, 2:KT, :])
                nc.sync.dma_start(triu[:], triu_in.ap())
                kq_half(8, 0, w=w8)
                for m in range(9, 16):
                    kq_half(m, 0)
                for m in range(0, 8):
                    kq_half(m, 0)

                # Late constants: emitted after the hp0 K/Q DMAs so they
                # don't delay the first matmul.
                nc.sync.dma_start(ident[:], ident_in.ap())
                for i in range(2):
                    nc.sync.dma_start(wp_sb[i][:], wproj.ap()[i])

                # Fillers woven through hp0's score stream: hp1 K/Q
                # halves, V chunks, then V transposes for pair 0.  Paced by
                # a PE-time budget (us) so scores arrive at the ~1.15us/blk
                # exp rate -- a fixed 1:1 weave starves the ACT exp stream
                # behind the 0.9-1.7us chunk fillers (measured 2-4.5us exp
                # waits).
                fillers = deque()
                for m in range(8, 16):
                    fillers.append((0.87, lambda m=m: kq_half(m, 1)))
                for m in range(0, 8):
                    fillers.append((0.87, lambda m=m: kq_half(m, 1)))
                for m in range(16, 24):
                    fillers.append((1.73, lambda m=m: v_chunk(m)))
                for jj in range(8):
                    fillers.append((0.22, lambda jj=jj: vtrans2(0, jj)))
                fillers.append((0.07, lambda: vnat_ones(0)))

                # B: hp0 scores 0..33, budget-paced fillers.
                budget = 0.0
                for i, (g, kb) in enumerate(blocks[:34]):
                    ex0[(g, kb)] = score_block(0, g, kb)
                    budget += 0.93
                    while fillers and budget >= fillers[0][0]:
                        cost, fn = fillers.popleft()
                        fn()
                        budget -= cost
                # any stragglers must land before C's AV blocks (vnat dep)
                while fillers:
                    fillers.popleft()[1]()

                # C: last hp0 scores + first hp0 AV groups (g0, g1).
                av0 = [avps.tile([65, 512], F32, tag="av",
                                 name=f"av0_0_{i}") for i in range(2)]
                rest = deque(blocks[34:])
                hp1_scores = deque(blocks)
                avq = deque()
                for g in range(2):
                    for kb in range(4 * g + 4):
                        avq.append((g, kb))
                while rest or avq:
                    if rest:
                        g, kb = rest.popleft()
                        ex0[(g, kb)] = score_block(0, g, kb)
                    for _ in range(2):
                        if avq:
                            g, kb = avq.popleft()
                            av_block(0, g, kb, ex0.pop((g, kb)), av0,
                                     4 * g + 4)
                            if kb == 4 * g + 3:
                                norm_g(0, g, av0)
                                if g == 0:
                                    av0 = [avps.tile([65, 512], F32, tag="av",
                                                     name=f"av0_1_{i}")
                                           for i in range(2)]

                # D: hp0 AV g2/g3 interleaved with hp1 scores; vtrans for
                # pair 0 blocks 8-15 first (needed by g2/g3), pair 1 after.
                fillers = deque()
                for jj in range(8, 16):
                    fillers.append(lambda jj=jj: vtrans2(0, jj))
                for jj in range(16):
                    fillers.append(lambda jj=jj: vtrans2(1, jj))
                fillers.append(lambda: vnat_ones(1))

                for g in range(2, NG):
                    av0 = [avps.tile([65, 512], F32, tag="av",
                                     name=f"av0_{g}_{i}") for i in range(2)]
                    for kb in range(4 * g + 4):
                        if fillers:
                            fillers.popleft()()
                        av_block(0, g, kb, ex0.pop((g, kb)), av0, 4 * g + 4)
                        if hp1_scores:
                            gg, kbb = hp1_scores.popleft()
                            ex1[(gg, kbb)] = score_block(1, gg, kbb)
                    norm_g(0, g, av0)

                # E: hp1 AVs per group, remaining hp1 scores just-in-time,
                # proj(g) right after norm(1, g).
                for g in range(NG):
                    av1 = [avps.tile([65, 512], F32, tag="av",
                                     name=f"av1_{g}_{i}") for i in range(2)]
                    for kb in range(4 * g + 4):
                        if fillers:
                            fillers.popleft()()
                        if (g, kb) not in ex1:
                            gg, kbb = hp1_scores.popleft()
                            ex1[(gg, kbb)] = score_block(1, gg, kbb)
                        av_block(1, g, kb, ex1.pop((g, kb)), av1, 4 * g + 4)
                        if hp1_scores:
                            gg, kbb = hp1_scores.popleft()
                            ex1[(gg, kbb)] = score_block(1, gg, kbb)
                    # proj delayed one group: proj(g-1)'s res is long
                    # ready, so the PE stays busy while norm(1,g)'s DVE
                    # chain runs (otherwise the boundary stall re-throttles
                    # the PE).  Final group: chain first, proj(g-1) under
                    # it, then per-block muls each feeding its proj block.
                    if g < NG - 1:
                        norm_g(1, g, av1)
                        if g > 0:
                            for blk in range(4 * g - 4, 4 * g):
                                proj_blk(blk, "dve")
                    else:
                        avsT, bcsT = norm_tail_chain(1, g, av1)
                        for blk in range(4 * g - 4, 4 * g):
                            proj_blk(blk, "split")
                        norm_tail_muls(1, g, avsT, bcsT)

                if DEBUG_TAPS:
                    nc.sync.dma_start(dbg_qt.ap(), qt[:])
                    nc.sync.dma_start(dbg_kt.ap(), kt_[:])
                    nc.sync.dma_start(dbg_vt.ap(), vt2[:])
                    for i in range(2):
                        nc.sync.dma_start(dbg_res.ap()[i], res[i][:])

    nc.compile()
    return nc


_CACHE = {}


def _get_program(with_qkv_bias: bool):
    if with_qkv_bias not in _CACHE:
        _CACHE[with_qkv_bias] = build_program(with_qkv_bias)
    return _CACHE[with_qkv_bias]


def make_in_maps(x, W_qkv, b_qkv, W_proj):
    """Build the 8 per-core input maps (host-side data marshaling only)."""
    x = np.ascontiguousarray(np.asarray(x, dtype=np.float32))
    W_qkv = np.asarray(W_qkv, dtype=np.float32)
    b_qkv = np.asarray(b_qkv, dtype=np.float32)
    W_proj = np.asarray(W_proj, dtype=np.float32)

    wq_t = np.ascontiguousarray(
        W_qkv.astype(ml_dtypes.bfloat16).reshape(KT, 128, NM, 128)
        .transpose(2, 1, 0, 3))
    wp_b = W_proj.astype(ml_dtypes.bfloat16)
    ident = np.eye(128).astype(ml_dtypes.bfloat16)
    # causal mask for diagonal blocks in sigma-slot order: visible k<=q
    perm = slot_perm()
    triu = (perm[:, None] <= perm[None, :]).astype(ml_dtypes.bfloat16)
    with_bias = bool(np.any(b_qkv))
    bias_t = np.ascontiguousarray(b_qkv.reshape(NM, 128).T) if with_bias else None

    in_maps = []
    for c in range(N_CORES):
        b, qi = c // 4, c % 4
        xc = x[b, ROWS * qi:ROWS * qi + ROWS, :]  # [512 rows, 1024]
        # xT[p, j, r] = xc[r, 128j+p], bf16
        xt = np.ascontiguousarray(
            xc.T.reshape(KT, 128, ROWS).transpose(1, 0, 2)
            .astype(ml_dtypes.bfloat16))
        m = {
            "xt": xt,
            "wqkv": wq_t,
            "wproj": np.ascontiguousarray(
                wp_b[256 * qi:256 * qi + 256, :].reshape(2, 128, E)),
            "ident": ident,
            "triu": triu,
        }
        if with_bias:
            m["bqkv"] = bias_t
        in_maps.append(m)
    return in_maps, with_bias


def kernel(x, W_qkv, b_qkv, W_proj, b_proj, _run_kwargs=None):
    in_maps, with_bias = make_in_maps(x, W_qkv, b_qkv, W_proj)
    nc = _get_program(with_bias)
    res = bass_utils.run_bass_kernel_spmd(
        nc, in_maps, core_ids=list(range(N_CORES)), **(_run_kwargs or {}))
    acc = np.zeros((B, S, E), np.float32)
    for c in range(N_CORES):
        acc[c // 4] += np.asarray(res.results[c]["out"], dtype=np.float32)
    # un-permute sequence rows (sigma-slot -> natural) within each 128-block
    perm = slot_perm()
    out = np.empty_like(acc)
    out.reshape(B, NB, 128, E)[:, :, perm, :] = acc.reshape(B, NB, 128, E)
    out += np.asarray(b_proj, dtype=np.float32)[None, None, :]
    if _run_kwargs:
        kernel.last_results = res
    return out
